# revision 9
# baseline (speedup 1.0000x reference)
"""Trainium2 Bass kernel for nn_DKF (deep Kalman filter inference).

Strategy (data-parallel over batch, 256 -> 32 per core on 8 cores):
  Everything lives in a "transposed" layout: feature dims on the 128 SBUF
  partitions, (time, batch) on the free axis.  Only the truly sequential
  work runs in the T=512 step loop:
    phase A (per 16-step group): xw = x @ W_ih + b as N=512 matmuls
    phase B (per step): RNN relu recurrence, 16 [128x128]x[128x32] matmuls
    phase C (per step): combiner tanh / mu / lv / reparam z, 12 matmuls
  The transition + emitter MLPs and the KL / reconstruction losses only
  depend on the stored z / mu / lv sequences, so they run as batched
  N=256 matmuls (phase D), interleaved per group so the elementwise work
  hides under the PE-bound step loop.  The two scalar losses are reduced
  on-device to per-partition partial sums and finished on the host.

  All matmul operands are bf16 (fp32 PSUM accumulation); mu / lv and the
  KL chain stay fp32.  sigmoid(x) is computed as 0.5*tanh(x/2)+0.5 so
  every activation used lives in the single "exp_and_others" ACT table.
"""
import sys
import os

sys.path.insert(0, "/opt/trn_rl_repo")

import numpy as np
import ml_dtypes
from contextlib import ExitStack

import concourse.bass as bass
import concourse.tile as tile
from concourse import bacc, mybir

BF = ml_dtypes.bfloat16
DT_BF = mybir.dt.bfloat16
DT_F32 = mybir.dt.float32
AF = mybir.ActivationFunctionType
ALU = mybir.AluOpType
AX = mybir.AxisListType

P = 128
B_FULL, T_FULL, D = 256, 512, 128
Z, TR, EM, RH = 128, 256, 256, 512
NCORES = 8
BC = B_FULL // NCORES  # 32

# ---- packed weight block offsets (units of 128 columns) ----
OFF = dict(wih=0, whh=4, cbh=20, cbmu=24, cblv=28, g1=32, g2=34,
           p1=36, p2=38, tmu=40, tlv=41, eh1=42, eh2=44, emu=48)
NWBLK = 50

# ---- packed bias columns ----
BCOL = dict(brnn=0, bg1=4, hbg2=6, bp2=7, bp1=8, btmu=10, btlv=11,
            hbcblv=12, bcbmu=13, bcblv=14, beh1=15, beh2=17, bemu=19,
            escale=20)
NBIAS = 21


def _wblk(w):
    return OFF[w] * 128


def _bcol(bp, name, i=0):
    c = BCOL[name] + i
    return bp[:, c:c + 1]


# ------------------------------------------------------------------
# host-side packing
# ------------------------------------------------------------------

def _pack_weights(ins):
    f = lambda a: np.asarray(a, np.float32)

    def blocks(W, nk, nm):
        return [W[k * 128:(k + 1) * 128, m * 128:(m + 1) * 128]
                for k in range(nk) for m in range(nm)]

    cols = []
    cols += blocks(f(ins["rnn_Wih"]), 1, 4)
    cols += blocks(f(ins["rnn_Whh"]), 4, 4)           # index k*4+m
    cols += blocks(f(ins["cb_h_W"]), 1, 4)
    cols += blocks(0.5 * f(ins["cb_mu_W"]), 4, 1)
    cols += blocks(0.5 * f(ins["cb_lv_W"]), 4, 1)
    cols += blocks(f(ins["tr_g1_W"]), 1, 2)
    cols += blocks(f(ins["tr_g2_W"]), 2, 1)
    cols += blocks(f(ins["tr_p1_W"]), 1, 2)
    cols += blocks(f(ins["tr_p2_W"]), 2, 1)
    cols += blocks(f(ins["tr_mu_W"]), 1, 1)
    cols += blocks(f(ins["tr_lv_W"]), 1, 1)
    cols += blocks(f(ins["em_h1_W"]), 1, 2)
    cols += blocks(f(ins["em_h2_W"]), 2, 2)           # index k*2+m
    cols += blocks(f(ins["em_mu_W"]), 2, 1)
    out = np.concatenate(cols, axis=1)
    assert out.shape == (P, NWBLK * 128), out.shape
    return out.astype(BF)


def _pack_biases(ins):
    f = lambda a: np.asarray(a, np.float32)
    bp = np.zeros((P, NBIAS), np.float32)

    def put(name, vec, nblk):
        v = f(vec)
        for m in range(nblk):
            bp[:, BCOL[name] + m] = v[m * 128:(m + 1) * 128]

    put("brnn", f(ins["rnn_bih"]) + f(ins["rnn_bhh"]), 4)
    put("bg1", ins["tr_g1_b"], 2)
    put("hbg2", 0.5 * f(ins["tr_g2_b"]), 1)
    put("bp2", ins["tr_p2_b"], 1)
    put("bp1", ins["tr_p1_b"], 2)
    put("btmu", ins["tr_mu_b"], 1)
    put("btlv", ins["tr_lv_b"], 1)
    put("hbcblv", 0.5 * f(ins["cb_lv_b"]), 1)
    put("bcbmu", ins["cb_mu_b"], 1)
    put("bcblv", ins["cb_lv_b"], 1)
    put("beh1", ins["em_h1_b"], 2)
    put("beh2", ins["em_h2_b"], 2)
    put("bemu", ins["em_mu_b"], 1)
    put("escale", np.exp(0.5 * f(ins["em_logvar"])), 1)
    return bp


def _bfull(ins):
    # [128, 128] fp32, col m*32+b = cb_h_b[m*128+p]
    v = np.asarray(ins["cb_h_b"], np.float32)
    out = np.zeros((P, 128), np.float32)
    for m in range(4):
        out[:, m * 32:(m + 1) * 32] = v[m * 128:(m + 1) * 128][:, None]
    return out


def _h0t(ins):
    v = np.asarray(ins["h0"], np.float32)
    out = np.zeros((P, 128), np.float32)
    for k in range(4):
        out[:, k * 32:(k + 1) * 32] = v[k * 128:(k + 1) * 128][:, None]
    return out.astype(BF)


# ------------------------------------------------------------------
# device program
# ------------------------------------------------------------------

def build_program(T=T_FULL, n_cores=NCORES, dbg=False):
    G = T // 16
    ND = T * BC
    nc = bacc.Bacc("TRN2", target_bir_lowering=False, debug=False,
                   num_devices=n_cores)
    dbg_aps = {}
    if dbg:
        dbg_aps["zal"] = nc.dram_tensor(
            "dbg_zal", [P, (T + 1) * 32], DT_BF, kind="ExternalOutput").ap()
        dbg_aps["d2"] = nc.dram_tensor(
            "dbg_d2", [P, 256], DT_BF, kind="ExternalOutput").ap()
        dbg_aps["xmu"] = nc.dram_tensor(
            "dbg_xmu", [P, 256], DT_BF, kind="ExternalOutput").ap()
        dbg_aps["re2"] = nc.dram_tensor(
            "dbg_re2", [P, 512], DT_BF, kind="ExternalOutput").ap()
        dbg_aps["stg"] = nc.dram_tensor(
            "dbg_stg", [P, 2048], DT_F32, kind="ExternalOutput").ap()
        dbg_aps["reS"] = nc.dram_tensor(
            "dbg_reS", [P, 2 * (T // 16)], DT_F32, kind="ExternalOutput").ap()
        dbg_aps["klS"] = nc.dram_tensor(
            "dbg_klS", [P, 2 * (T // 16)], DT_F32, kind="ExternalOutput").ap()
        dbg_aps["esx"] = nc.dram_tensor(
            "dbg_esx", [P, 256], DT_BF, kind="ExternalOutput").ap()
        dbg_aps["ytd"] = nc.dram_tensor(
            "dbg_ytd", [P, 256], DT_BF, kind="ExternalOutput").ap()
        dbg_aps["d1"] = nc.dram_tensor(
            "dbg_d1", [P, 256], DT_BF, kind="ExternalOutput").ap()

    xT = nc.dram_tensor("xT", [P, ND], DT_BF, kind="ExternalInput").ap()
    yT = nc.dram_tensor("yT", [P, ND], DT_BF, kind="ExternalInput").ap()
    epsC = nc.dram_tensor("epsC", [P, ND], DT_BF, kind="ExternalInput").ap()
    epsE = nc.dram_tensor("epsE", [P, ND], DT_BF, kind="ExternalInput").ap()
    wpk = nc.dram_tensor("wpk", [P, NWBLK * 128], DT_BF, kind="ExternalInput").ap()
    bpk = nc.dram_tensor("bpk", [P, NBIAS], DT_F32, kind="ExternalInput").ap()
    bfu = nc.dram_tensor("bfu", [P, 128], DT_F32, kind="ExternalInput").ap()
    h0d = nc.dram_tensor("h0t", [P, 128], DT_BF, kind="ExternalInput").ap()
    z0d = nc.dram_tensor("z0t", [P, 32], DT_BF, kind="ExternalInput").ap()
    out_d = nc.dram_tensor("res", [1, 2], DT_F32, kind="ExternalOutput").ap()

    with ExitStack() as ctx:
        tc = ctx.enter_context(tile.TileContext(nc))
        const = ctx.enter_context(tc.tile_pool(name="const", bufs=1))
        xs = ctx.enter_context(tc.tile_pool(name="xs", bufs=3))
        esp = ctx.enter_context(tc.tile_pool(name="esp", bufs=3))
        dstr = ctx.enter_context(tc.tile_pool(name="dstr", bufs=3))
        wk = ctx.enter_context(tc.tile_pool(name="wk", bufs=3))
        wkd = ctx.enter_context(tc.tile_pool(name="wkd", bufs=2))
        psb = ctx.enter_context(tc.tile_pool(name="psb", bufs=2, space="PSUM"))
        psh = ctx.enter_context(tc.tile_pool(name="psh", bufs=1, space="PSUM"))
        psm = ctx.enter_context(tc.tile_pool(name="psm", bufs=1, space="PSUM"))
        pda = ctx.enter_context(tc.tile_pool(name="pda", bufs=1, space="PSUM"))
        pdb = ctx.enter_context(tc.tile_pool(name="pdb", bufs=2, space="PSUM"))

        wt = const.tile([P, NWBLK * 128], DT_BF)
        nc.sync.dma_start(wt[:], wpk)
        bp = const.tile([P, NBIAS], DT_F32)
        nc.sync.dma_start(bp[:], bpk)
        bfl = const.tile([P, 128], DT_F32)
        nc.sync.dma_start(bfl[:], bfu)
        h0 = const.tile([P, 128], DT_BF)
        nc.sync.dma_start(h0[:], h0d)
        zal = const.tile([P, (T + 1) * 32], DT_BF)
        nc.sync.dma_start(zal[:, 0:32], z0d)
        xwr = const.tile([P, 2 * 2048], DT_BF)
        hr = const.tile([P, 8 * 128], DT_BF)
        stg = const.tile([P, 2 * 1024], DT_F32)
        klS = const.tile([P, 2 * G], DT_F32)
        reS = const.tile([P, 2 * G], DT_F32)
        fin = const.tile([1, 2], DT_F32)

        xw3 = xwr[:].rearrange("p (sl m s b) -> p sl m s b",
                               sl=2, m=4, s=16, b=32)

        def wtb(name, i=0):
            c = _wblk(name) + i * 128
            return wt[:, c:c + 128]

        def d_chunk(n, slot, hf):
            c0 = n * 256
            zp = zal[:, c0:c0 + 256]
            zc = zal[:, c0 + 32:c0 + 288]
            mu_t = stg[:, slot * 1024 + hf * 256: slot * 1024 + hf * 256 + 256]
            lv_t = stg[:, slot * 1024 + 512 + hf * 256:
                       slot * 1024 + 512 + hf * 256 + 256]
            y_t = dstr.tile([P, 256], DT_BF, tag="yt")
            nc.sync.dma_start(y_t[:], yT[:, c0:c0 + 256])
            ee_t = dstr.tile([P, 256], DT_BF, tag="ee")
            nc.sync.dma_start(ee_t[:], epsE[:, c0:c0 + 256])

            # ---- transition ----
            t1 = pda.tile([P, 1024], DT_F32, tag="big")
            for m in range(2):
                nc.tensor.matmul(t1[:, m * 256:(m + 1) * 256], wtb("g1", m),
                                 zp, start=True, stop=True)
            for m in range(2):
                nc.tensor.matmul(t1[:, 512 + m * 256:512 + (m + 1) * 256],
                                 wtb("p1", m), zp, start=True, stop=True)
            r1 = wkd.tile([P, 1024], DT_BF, tag="r1")
            nc.scalar.activation(r1[:, 0:256], t1[:, 0:256], AF.Relu,
                                 bias=_bcol(bp, "bg1", 0))
            nc.scalar.activation(r1[:, 256:512], t1[:, 256:512], AF.Relu,
                                 bias=_bcol(bp, "bg1", 1))
            nc.vector.tensor_scalar(r1[:, 512:768], t1[:, 512:768],
                                    _bcol(bp, "bp1", 0), 0.0,
                                    op0=ALU.add, op1=ALU.max)
            nc.vector.tensor_scalar(r1[:, 768:1024], t1[:, 768:1024],
                                    _bcol(bp, "bp1", 1), 0.0,
                                    op0=ALU.add, op1=ALU.max)
            t2 = pdb.tile([P, 512], DT_F32, tag="med")
            for k in range(2):
                nc.tensor.matmul(t2[:, 0:256], wtb("g2", k),
                                 r1[:, k * 256:(k + 1) * 256],
                                 start=(k == 0), stop=(k == 1))
            for k in range(2):
                nc.tensor.matmul(t2[:, 256:512], wtb("p2", k),
                                 r1[:, 512 + k * 256:512 + (k + 1) * 256],
                                 start=(k == 0), stop=(k == 1))
            gate = wkd.tile([P, 256], DT_BF, tag="gate")
            nc.scalar.activation(gate[:], t2[:, 0:256], AF.Tanh,
                                 bias=_bcol(bp, "hbg2"), scale=0.5)
            prop = wkd.tile([P, 256], DT_F32, tag="prop")
            nc.vector.tensor_scalar_add(prop[:], t2[:, 256:512],
                                        _bcol(bp, "bp2"))
            rp2 = wkd.tile([P, 256], DT_BF, tag="rp2")
            nc.vector.tensor_scalar(rp2[:], t2[:, 256:512], _bcol(bp, "bp2"),
                                    0.0, op0=ALU.add, op1=ALU.max)
            t3 = pdb.tile([P, 512], DT_F32, tag="med")
            nc.tensor.matmul(t3[:, 0:256], wtb("tmu"), zp, start=True, stop=True)
            nc.tensor.matmul(t3[:, 256:512], wtb("tlv"), rp2[:],
                             start=True, stop=True)
            mub = wkd.tile([P, 256], DT_F32, tag="mub")
            nc.vector.tensor_scalar_add(mub[:], t3[:, 0:256], _bcol(bp, "btmu"))
            prlv = wkd.tile([P, 256], DT_F32, tag="prlv")
            nc.vector.tensor_scalar_add(prlv[:], t3[:, 256:512],
                                        _bcol(bp, "btlv"))
            # pr_mu = mub - 0.5*(dd + gate*dd),  dd = mub - prop
            dd = wkd.tile([P, 256], DT_F32, tag="dd")
            nc.gpsimd.tensor_tensor(dd[:], mub[:], prop[:], op=ALU.subtract)
            td = wkd.tile([P, 256], DT_F32, tag="td")
            nc.vector.tensor_tensor(td[:], gate[:], dd[:], op=ALU.mult)
            dts = wkd.tile([P, 256], DT_F32, tag="dts")
            nc.gpsimd.tensor_tensor(dts[:], dd[:], td[:], op=ALU.add)
            prmu = wkd.tile([P, 256], DT_F32, tag="prmu")
            nc.vector.scalar_tensor_tensor(prmu[:], dts[:], -0.5, mub[:],
                                           op0=ALU.mult, op1=ALU.add)
            # KL pieces: kl_elt = exp(lv-prlv) + (mu-prmu)^2*exp(-prlv) - (lv-prlv)
            a_ = wkd.tile([P, 256], DT_F32, tag="a")
            nc.gpsimd.tensor_tensor(a_[:], lv_t, prlv[:], op=ALU.subtract)
            e1 = wkd.tile([P, 256], DT_F32, tag="e1")
            nc.scalar.activation(e1[:], a_[:], AF.Exp)
            en = wkd.tile([P, 256], DT_F32, tag="en")
            nc.scalar.activation(en[:], prlv[:], AF.Exp, scale=-1.0)
            c_ = wkd.tile([P, 256], DT_F32, tag="c")
            nc.gpsimd.tensor_tensor(c_[:], mu_t, prmu[:], op=ALU.subtract)
            c2 = wkd.tile([P, 256], DT_F32, tag="c2")
            nc.gpsimd.tensor_tensor(c2[:], c_[:], c_[:], op=ALU.mult)
            t4 = wkd.tile([P, 256], DT_F32, tag="t4")
            nc.vector.tensor_tensor(t4[:], c2[:], en[:], op=ALU.mult)
            e1t = wkd.tile([P, 256], DT_F32, tag="e1t")
            nc.gpsimd.tensor_tensor(e1t[:], e1[:], t4[:], op=ALU.add)
            kscr = wkd.tile([P, 256], DT_F32, tag="kscr")
            nc.vector.scalar_tensor_tensor(kscr[:], e1t[:], 0.0, a_[:],
                                           op0=ALU.add, op1=ALU.subtract,
                                           accum_out=klS[:, n:n + 1])
            # ---- emitter + rec ----
            e12 = pda.tile([P, 1024], DT_F32, tag="big")
            for m in range(2):
                nc.tensor.matmul(e12[:, m * 256:(m + 1) * 256], wtb("eh1", m),
                                 zc, start=True, stop=True)
            re1 = wkd.tile([P, 512], DT_BF, tag="re1")
            nc.scalar.activation(re1[:, 0:256], e12[:, 0:256], AF.Relu,
                                 bias=_bcol(bp, "beh1", 0))
            nc.scalar.activation(re1[:, 256:512], e12[:, 256:512], AF.Relu,
                                 bias=_bcol(bp, "beh1", 1))
            for m in range(2):
                for k in range(2):
                    nc.tensor.matmul(e12[:, 512 + m * 256:512 + (m + 1) * 256],
                                     wtb("eh2", k * 2 + m),
                                     re1[:, k * 256:(k + 1) * 256],
                                     start=(k == 0), stop=(k == 1))
            re2 = wkd.tile([P, 512], DT_BF, tag="re2")
            nc.scalar.activation(re2[:, 0:256], e12[:, 512:768], AF.Relu,
                                 bias=_bcol(bp, "beh2", 0))
            nc.vector.tensor_scalar(re2[:, 256:512], e12[:, 768:1024],
                                    _bcol(bp, "beh2", 1), 0.0,
                                    op0=ALU.add, op1=ALU.max)
            e3 = pdb.tile([P, 512], DT_F32, tag="med")
            for k in range(2):
                nc.tensor.matmul(e3[:, 0:256], wtb("emu", k),
                                 re2[:, k * 256:(k + 1) * 256],
                                 start=(k == 0), stop=(k == 1))
            xmu = wkd.tile([P, 256], DT_BF, tag="xmu")
            nc.scalar.activation(xmu[:], e3[:, 0:256], AF.Identity,
                                 bias=_bcol(bp, "bemu"))
            esx = wkd.tile([P, 256], DT_BF, tag="esx")
            nc.vector.tensor_scalar(esx[:], ee_t[:], _bcol(bp, "escale"), None,
                                    op0=ALU.mult)
            d1 = wkd.tile([P, 256], DT_BF, tag="d1")
            nc.vector.tensor_tensor(d1[:], xmu[:], esx[:], op=ALU.add)
            d2 = wkd.tile([P, 256], DT_BF, tag="d2")
            nc.vector.tensor_tensor(d2[:], d1[:], y_t[:], op=ALU.subtract)
            rscr = wkd.tile([P, 256], DT_F32, tag="rscr")
            nc.scalar.activation(rscr[:], d2[:], AF.Square,
                                 accum_out=reS[:, n:n + 1])
            if dbg and n == 0:
                nc.sync.dma_start(dbg_aps["ytd"], y_t[:])
                nc.sync.dma_start(dbg_aps["d1"], d1[:])
                nc.sync.dma_start(dbg_aps["esx"], esx[:])
                nc.sync.dma_start(dbg_aps["d2"], d2[:])
                nc.sync.dma_start(dbg_aps["xmu"], xmu[:])
                nc.sync.dma_start(dbg_aps["re2"], re2[:])

        for g in range(G):
            slot = g % 2
            xc = xs.tile([P, 512], DT_BF, tag="xc")
            nc.sync.dma_start(xc[:], xT[:, g * 512:(g + 1) * 512])
            ec = esp.tile([P, 512], DT_BF, tag="ec")
            nc.sync.dma_start(ec[:], epsC[:, g * 512:(g + 1) * 512])
            # ---- phase A: xw for this group ----
            for m in range(4):
                pa = pdb.tile([P, 512], DT_F32, tag="med")
                nc.tensor.matmul(pa[:], wtb("wih", m), xc[:],
                                 start=True, stop=True)
                dst = xwr[:, slot * 2048 + m * 512: slot * 2048 + (m + 1) * 512]
                if m % 2 == 0:
                    nc.scalar.activation(dst, pa[:], AF.Identity,
                                         bias=_bcol(bp, "brnn", m))
                else:
                    nc.vector.tensor_scalar_add(dst, pa[:],
                                                _bcol(bp, "brnn", m))
            # ---- 16 steps of B and C ----
            for s in range(16):
                t = 16 * g + s
                hs_ = t % 8
                hp = (t - 1) % 8
                # B(t): h = relu(xw_t + h_prev @ Whh)
                pb = psb.tile([P, 128], DT_F32, tag="b")
                for m in range(4):
                    for k in range(4):
                        if t == 0:
                            rhs = h0[:, k * 32:(k + 1) * 32]
                        else:
                            rhs = hr[:, hp * 128 + k * 32: hp * 128 + k * 32 + 32]
                        nc.tensor.matmul(pb[:, m * 32:(m + 1) * 32],
                                         wtb("whh", k * 4 + m), rhs,
                                         start=(k == 0), stop=(k == 3))
                tmpb = wk.tile([P, 128], DT_F32, tag="tmpb")
                nc.vector.tensor_tensor(
                    tmpb[:].rearrange("p (m b) -> p m b", m=4),
                    pb[:].rearrange("p (m b) -> p m b", m=4),
                    xw3[:, slot, :, s, :], op=ALU.add)
                nc.scalar.activation(hr[:, hs_ * 128:(hs_ + 1) * 128],
                                     tmpb[:], AF.Relu)
                # C(t): combiner + reparam
                ph = psh.tile([P, 128], DT_F32, tag="hc")
                zprev = zal[:, t * 32:(t + 1) * 32]
                for m in range(4):
                    nc.tensor.matmul(ph[:, m * 32:(m + 1) * 32],
                                     wtb("cbh", m), zprev,
                                     start=True, stop=True)
                tmp1 = wk.tile([P, 128], DT_F32, tag="tmp1")
                nc.vector.tensor_tensor(tmp1[:], ph[:], bfl[:], op=ALU.add)
                tmp2 = wk.tile([P, 128], DT_BF, tag="tmp2")
                nc.scalar.activation(tmp2[:], tmp1[:], AF.Tanh)
                hc2 = wk.tile([P, 128], DT_BF, tag="hc2")
                nc.vector.tensor_tensor(hc2[:], tmp2[:],
                                        hr[:, hs_ * 128:(hs_ + 1) * 128],
                                        op=ALU.add)
                pm = psm.tile([P, 64], DT_F32, tag="ml")
                for k in range(4):
                    nc.tensor.matmul(pm[:, 0:32], wtb("cbmu", k),
                                     hc2[:, k * 32:(k + 1) * 32],
                                     start=(k == 0), stop=(k == 3))
                for k in range(4):
                    nc.tensor.matmul(pm[:, 32:64], wtb("cblv", k),
                                     hc2[:, k * 32:(k + 1) * 32],
                                     start=(k == 0), stop=(k == 3))
                ex = wk.tile([P, 32], DT_F32, tag="ex")
                nc.scalar.activation(ex[:], pm[:, 32:64], AF.Exp,
                                     bias=_bcol(bp, "hbcblv"), scale=0.5)
                mu_sl = stg[:, slot * 1024 + s * 32: slot * 1024 + s * 32 + 32]
                nc.scalar.activation(mu_sl, pm[:, 0:32], AF.Identity,
                                     bias=_bcol(bp, "bcbmu"))
                lv_sl = stg[:, slot * 1024 + 512 + s * 32:
                            slot * 1024 + 512 + s * 32 + 32]
                nc.scalar.activation(lv_sl, pm[:, 32:64], AF.Identity,
                                     bias=_bcol(bp, "bcblv"))
                t2t = wk.tile([P, 32], DT_F32, tag="t2t")
                nc.vector.tensor_tensor(t2t[:], ex[:],
                                        ec[:, s * 32:(s + 1) * 32],
                                        op=ALU.mult)
                nc.vector.tensor_tensor(zal[:, (t + 1) * 32:(t + 2) * 32],
                                        t2t[:], mu_sl, op=ALU.add)
            # ---- phase D for this group ----
            d_chunk(2 * g, slot, 0)
            d_chunk(2 * g + 1, slot, 1)

        # ---- final reduction ----
        sums2 = const.tile([P, 2], DT_F32)
        nc.vector.tensor_reduce(sums2[:, 0:1], reS[:], axis=AX.X, op=ALU.add)
        nc.vector.tensor_reduce(sums2[:, 1:2], klS[:], axis=AX.X, op=ALU.add)
        ones = const.tile([P, 1], DT_F32)
        nc.vector.memset(ones[:], 1.0)
        psf = psm.tile([1, 2], DT_F32, tag="ml")
        nc.tensor.matmul(psf[:], ones[:], sums2[:], start=True, stop=True)
        nc.vector.tensor_copy(fin[:], psf[:])
        nc.sync.dma_start(out_d, fin[:])
        if dbg:
            nc.sync.dma_start(dbg_aps["zal"], zal[:])
            nc.sync.dma_start(dbg_aps["stg"], stg[:])
            nc.sync.dma_start(dbg_aps["reS"], reS[:])
            nc.sync.dma_start(dbg_aps["klS"], klS[:])

    nc.compile()
    return nc


# ------------------------------------------------------------------
# host wrapper
# ------------------------------------------------------------------

def make_in_maps(inputs, T=T_FULL, n_cores=NCORES):
    wpk = _pack_weights(inputs)
    bpk = _pack_biases(inputs)
    bfu = _bfull(inputs)
    h0t = _h0t(inputs)
    z0t = np.broadcast_to(
        np.asarray(inputs["zq0"], np.float32)[:, None], (P, 32)
    ).astype(BF).copy()

    x = np.asarray(inputs["x"])[:, :T]
    y = np.asarray(inputs["y"])[:, :T]
    eC = np.asarray(inputs["eps_comb"])[:T]
    eE = np.asarray(inputs["eps_emit"])[:T]

    in_maps = []
    for c in range(n_cores):
        bs = slice(c * BC, (c + 1) * BC)
        xTc = np.ascontiguousarray(
            x[bs].transpose(2, 1, 0).reshape(P, T * BC)).astype(BF)
        yTc = np.ascontiguousarray(
            y[bs].transpose(2, 1, 0).reshape(P, T * BC)).astype(BF)
        eCc = np.ascontiguousarray(
            eC[:, bs, :].transpose(2, 0, 1).reshape(P, T * BC)).astype(BF)
        eEc = np.ascontiguousarray(
            eE[:, bs, :].transpose(2, 0, 1).reshape(P, T * BC)).astype(BF)
        in_maps.append(dict(xT=xTc, yT=yTc, epsC=eCc, epsE=eEc,
                            wpk=wpk, bpk=bpk, bfu=bfu, h0t=h0t, z0t=z0t))
    return in_maps


def combine_outputs(results, T=T_FULL, n_cores=NCORES):
    rec_sum = float(sum(r["res"][0, 0] for r in results))
    kl_sum = float(sum(r["res"][0, 1] for r in results))
    n_tb = T * BC * n_cores
    rec_loss = rec_sum / (n_tb * D)
    kl_loss = 0.5 * (kl_sum - n_tb * Z) / n_tb
    return np.array([rec_loss, kl_loss], np.float32)


_CACHE = {}


def kernel(**inputs):
    from concourse.bass_utils import run_bass_kernel_spmd
    T = T_FULL
    if T not in _CACHE:
        _CACHE[T] = build_program(T)
    nc = _CACHE[T]
    in_maps = make_in_maps(inputs, T)
    res = run_bass_kernel_spmd(nc, in_maps, core_ids=list(range(NCORES)))
    return combine_outputs(res.results, T)


if __name__ == "__main__":
    import jax
    inputs = None
    sys.path.insert(0, "/root/problem")
    import reference
    inputs = {k: np.asarray(v) for k, v in reference.setup_inputs().items()}
    out = kernel(**inputs)
    print("kernel out:", out)


# revision 19
# speedup vs baseline: 47.6944x; 47.6944x over previous
"""Trainium2 Bass kernel for nn_DKF (deep Kalman filter inference).

Strategy (data-parallel over batch, 256 -> 32 per core on 8 cores):
  Everything lives in a "transposed" layout: feature dims on the 128 SBUF
  partitions, (time, batch) on the free axis.  Only the truly sequential
  work runs in the T=512 step loop:
    phase A (per 16-step group): xw = x @ W_ih + b as N=512 matmuls
    phase B (per step): RNN relu recurrence, 16 [128x128]x[128x32] matmuls
    phase C (per step): combiner tanh / mu / lv / reparam z, 12 matmuls
  The transition + emitter MLPs and the KL / reconstruction losses only
  depend on the stored z / mu / lv sequences, so they run as batched
  N=256 matmuls (phase D), interleaved per group so the elementwise work
  hides under the PE-bound step loop.  The two scalar losses are reduced
  on-device to per-partition partial sums and finished on the host.

  All matmul operands are bf16 (fp32 PSUM accumulation); mu / lv and the
  KL chain stay fp32.  sigmoid(x) is computed as 0.5*tanh(x/2)+0.5 so
  every activation used lives in the single "exp_and_others" ACT table.
"""
import sys
import os

sys.path.insert(0, "/opt/trn_rl_repo")

import numpy as np
import ml_dtypes
from contextlib import ExitStack

import concourse.bass as bass
import concourse.tile as tile
from concourse import bacc, mybir

BF = ml_dtypes.bfloat16
DT_BF = mybir.dt.bfloat16
DT_F32 = mybir.dt.float32
AF = mybir.ActivationFunctionType
ALU = mybir.AluOpType
AX = mybir.AxisListType

P = 128
B_FULL, T_FULL, D = 256, 512, 128
Z, TR, EM, RH = 128, 256, 256, 512
NCORES = 8
BC = B_FULL // NCORES  # 32

# ---- packed weight block offsets (units of 128 columns) ----
OFF = dict(wih=0, whh=4, cbh=20, cbmu=24, cblv=28, g1=32, g2=34,
           p1=36, p2=38, tmu=40, tlv=41, eh1=42, eh2=44, emu=48)
NWBLK = 50

# ---- packed bias columns ----
BCOL = dict(brnn=0, bg1=4, hbg2=6, bp2=7, bp1=8, btmu=10, btlv=11,
            hbcblv=12, bcbmu=13, bcblv=14, beh1=15, beh2=17, bemu=19,
            escale=20)
NBIAS = 21


def _wblk(w):
    return OFF[w] * 128


def _bcol(bp, name, i=0):
    c = BCOL[name] + i
    return bp[:, c:c + 1]


# ------------------------------------------------------------------
# host-side packing
# ------------------------------------------------------------------

def _pack_weights(ins):
    f = lambda a: np.asarray(a, np.float32)

    def blocks(W, nk, nm):
        return [W[k * 128:(k + 1) * 128, m * 128:(m + 1) * 128]
                for k in range(nk) for m in range(nm)]

    cols = []
    cols += blocks(f(ins["rnn_Wih"]), 1, 4)
    cols += blocks(f(ins["rnn_Whh"]), 4, 4)           # index k*4+m
    cols += blocks(f(ins["cb_h_W"]), 1, 4)
    cols += blocks(0.5 * f(ins["cb_mu_W"]), 4, 1)
    cols += blocks(0.5 * f(ins["cb_lv_W"]), 4, 1)
    cols += blocks(f(ins["tr_g1_W"]), 1, 2)
    cols += blocks(f(ins["tr_g2_W"]), 2, 1)
    cols += blocks(f(ins["tr_p1_W"]), 1, 2)
    cols += blocks(f(ins["tr_p2_W"]), 2, 1)
    cols += blocks(f(ins["tr_mu_W"]), 1, 1)
    cols += blocks(f(ins["tr_lv_W"]), 1, 1)
    cols += blocks(f(ins["em_h1_W"]), 1, 2)
    cols += blocks(f(ins["em_h2_W"]), 2, 2)           # index k*2+m
    cols += blocks(f(ins["em_mu_W"]), 2, 1)
    out = np.concatenate(cols, axis=1)
    assert out.shape == (P, NWBLK * 128), out.shape
    return out.astype(BF)


def _pack_biases(ins):
    f = lambda a: np.asarray(a, np.float32)
    bp = np.zeros((P, NBIAS), np.float32)

    def put(name, vec, nblk):
        v = f(vec)
        for m in range(nblk):
            bp[:, BCOL[name] + m] = v[m * 128:(m + 1) * 128]

    put("brnn", f(ins["rnn_bih"]) + f(ins["rnn_bhh"]), 4)
    put("bg1", ins["tr_g1_b"], 2)
    put("hbg2", 0.5 * f(ins["tr_g2_b"]), 1)
    put("bp2", ins["tr_p2_b"], 1)
    put("bp1", ins["tr_p1_b"], 2)
    put("btmu", ins["tr_mu_b"], 1)
    put("btlv", ins["tr_lv_b"], 1)
    put("hbcblv", 0.5 * f(ins["cb_lv_b"]), 1)
    put("bcbmu", ins["cb_mu_b"], 1)
    put("bcblv", ins["cb_lv_b"], 1)
    put("beh1", ins["em_h1_b"], 2)
    put("beh2", ins["em_h2_b"], 2)
    put("bemu", ins["em_mu_b"], 1)
    put("escale", np.exp(0.5 * f(ins["em_logvar"])), 1)
    return bp


def _bfull(ins):
    # [128, 128] fp32, col m*32+b = cb_h_b[m*128+p]
    v = np.asarray(ins["cb_h_b"], np.float32)
    out = np.zeros((P, 128), np.float32)
    for m in range(4):
        out[:, m * 32:(m + 1) * 32] = v[m * 128:(m + 1) * 128][:, None]
    return out


def _h0t(ins):
    v = np.asarray(ins["h0"], np.float32)
    out = np.zeros((P, 128), np.float32)
    for k in range(4):
        out[:, k * 32:(k + 1) * 32] = v[k * 128:(k + 1) * 128][:, None]
    return out.astype(BF)


# ------------------------------------------------------------------
# device program
# ------------------------------------------------------------------

def build_program(T=T_FULL, n_cores=NCORES, dbg=False, reps=1,
                  psm_bufs=2, pdb_bufs=1, psh_bufs=1, d_split=16,
                  skip_cbh_bias=False, lv_first=True, stage_dve=False):
    G = T // 16
    ND = T * BC
    nc = bacc.Bacc("TRN2", target_bir_lowering=False, debug=False,
                   num_devices=n_cores)
    dbg_aps = {}
    if dbg:
        dbg_aps["zal"] = nc.dram_tensor(
            "dbg_zal", [P, (T + 1) * 32], DT_BF, kind="ExternalOutput").ap()
        dbg_aps["d2"] = nc.dram_tensor(
            "dbg_d2", [P, 256], DT_BF, kind="ExternalOutput").ap()
        dbg_aps["xmu"] = nc.dram_tensor(
            "dbg_xmu", [P, 256], DT_BF, kind="ExternalOutput").ap()
        dbg_aps["re2"] = nc.dram_tensor(
            "dbg_re2", [P, 512], DT_BF, kind="ExternalOutput").ap()
        dbg_aps["stg"] = nc.dram_tensor(
            "dbg_stg", [P, 2048], DT_F32, kind="ExternalOutput").ap()
        dbg_aps["reS"] = nc.dram_tensor(
            "dbg_reS", [P, 2 * (T // 16)], DT_F32, kind="ExternalOutput").ap()
        dbg_aps["klS"] = nc.dram_tensor(
            "dbg_klS", [P, 2 * (T // 16)], DT_F32, kind="ExternalOutput").ap()
        dbg_aps["esx"] = nc.dram_tensor(
            "dbg_esx", [P, 256], DT_BF, kind="ExternalOutput").ap()
        dbg_aps["ytd"] = nc.dram_tensor(
            "dbg_ytd", [P, 256], DT_BF, kind="ExternalOutput").ap()
        dbg_aps["d1"] = nc.dram_tensor(
            "dbg_d1", [P, 256], DT_BF, kind="ExternalOutput").ap()

    xT = nc.dram_tensor("xT", [P, ND], DT_BF, kind="ExternalInput").ap()
    yT = nc.dram_tensor("yT", [P, ND], DT_BF, kind="ExternalInput").ap()
    epsC = nc.dram_tensor("epsC", [P, ND], DT_BF, kind="ExternalInput").ap()
    epsE = nc.dram_tensor("epsE", [P, ND], DT_BF, kind="ExternalInput").ap()
    wpk = nc.dram_tensor("wpk", [P, NWBLK * 128], DT_BF, kind="ExternalInput").ap()
    bpk = nc.dram_tensor("bpk", [P, NBIAS], DT_F32, kind="ExternalInput").ap()
    bfu = nc.dram_tensor("bfu", [P, 128], DT_F32, kind="ExternalInput").ap()
    h0d = nc.dram_tensor("h0t", [P, 128], DT_BF, kind="ExternalInput").ap()
    z0d = nc.dram_tensor("z0t", [P, 32], DT_BF, kind="ExternalInput").ap()
    out_d = nc.dram_tensor("res", [1, 2], DT_F32, kind="ExternalOutput").ap()

    with ExitStack() as ctx:
        tc = ctx.enter_context(tile.TileContext(nc))
        const = ctx.enter_context(tc.tile_pool(name="const", bufs=1))
        xs = ctx.enter_context(tc.tile_pool(name="xs", bufs=3))
        esp = ctx.enter_context(tc.tile_pool(name="esp", bufs=3))
        dstr = ctx.enter_context(tc.tile_pool(name="dstr", bufs=3))
        wk = ctx.enter_context(tc.tile_pool(name="wk", bufs=3))
        wkd = ctx.enter_context(tc.tile_pool(name="wkd", bufs=2))
        psb = ctx.enter_context(tc.tile_pool(name="psb", bufs=2, space="PSUM"))
        psh = ctx.enter_context(tc.tile_pool(name="psh", bufs=psh_bufs, space="PSUM"))
        psm = ctx.enter_context(tc.tile_pool(name="psm", bufs=psm_bufs, space="PSUM"))
        pda = ctx.enter_context(tc.tile_pool(name="pda", bufs=1, space="PSUM"))
        pdb = ctx.enter_context(tc.tile_pool(name="pdb", bufs=pdb_bufs, space="PSUM"))

        wt = const.tile([P, NWBLK * 128], DT_BF)
        nc.sync.dma_start(wt[:], wpk)
        bp = const.tile([P, NBIAS], DT_F32)
        nc.sync.dma_start(bp[:], bpk)
        bfl = const.tile([P, 128], DT_F32)
        nc.sync.dma_start(bfl[:], bfu)
        h0 = const.tile([P, 128], DT_BF)
        nc.sync.dma_start(h0[:], h0d)
        zal = const.tile([P, (T + 1) * 32], DT_BF)
        nc.sync.dma_start(zal[:, 0:32], z0d)
        xwr = const.tile([P, 2 * 2048], DT_BF)
        hr = const.tile([P, 8 * 128], DT_BF)
        stg = const.tile([P, 2 * 1024], DT_F32)
        klS = const.tile([P, 2 * G], DT_F32)
        reS = const.tile([P, 2 * G], DT_F32)
        fin = const.tile([1, 2], DT_F32)

        xw3 = xwr[:].rearrange("p (sl m s b) -> p sl m s b",
                               sl=2, m=4, s=16, b=32)

        def wtb(name, i=0):
            c = _wblk(name) + i * 128
            return wt[:, c:c + 128]

        def d_chunk(n, slot, hf):
            c0 = n * 256
            zp = zal[:, c0:c0 + 256]
            zc = zal[:, c0 + 32:c0 + 288]
            mu_t = stg[:, slot * 1024 + hf * 256: slot * 1024 + hf * 256 + 256]
            lv_t = stg[:, slot * 1024 + 512 + hf * 256:
                       slot * 1024 + 512 + hf * 256 + 256]
            y_t = dstr.tile([P, 256], DT_BF, tag="yt")
            nc.sync.dma_start(y_t[:], yT[:, c0:c0 + 256])
            ee_t = dstr.tile([P, 256], DT_BF, tag="ee")
            nc.sync.dma_start(ee_t[:], epsE[:, c0:c0 + 256])

            # ---- transition ----
            t1 = pda.tile([P, 1024], DT_F32, tag="big")
            for m in range(2):
                nc.tensor.matmul(t1[:, m * 256:(m + 1) * 256], wtb("g1", m),
                                 zp, start=True, stop=True)
            for m in range(2):
                nc.tensor.matmul(t1[:, 512 + m * 256:512 + (m + 1) * 256],
                                 wtb("p1", m), zp, start=True, stop=True)
            r1 = wkd.tile([P, 1024], DT_BF, tag="r1")
            nc.scalar.activation(r1[:, 0:256], t1[:, 0:256], AF.Relu,
                                 bias=_bcol(bp, "bg1", 0))
            nc.scalar.activation(r1[:, 256:512], t1[:, 256:512], AF.Relu,
                                 bias=_bcol(bp, "bg1", 1))
            nc.vector.tensor_scalar(r1[:, 512:768], t1[:, 512:768],
                                    _bcol(bp, "bp1", 0), 0.0,
                                    op0=ALU.add, op1=ALU.max)
            nc.vector.tensor_scalar(r1[:, 768:1024], t1[:, 768:1024],
                                    _bcol(bp, "bp1", 1), 0.0,
                                    op0=ALU.add, op1=ALU.max)
            t2 = pdb.tile([P, 512], DT_F32, tag="med")
            for k in range(2):
                nc.tensor.matmul(t2[:, 0:256], wtb("g2", k),
                                 r1[:, k * 256:(k + 1) * 256],
                                 start=(k == 0), stop=(k == 1))
            for k in range(2):
                nc.tensor.matmul(t2[:, 256:512], wtb("p2", k),
                                 r1[:, 512 + k * 256:512 + (k + 1) * 256],
                                 start=(k == 0), stop=(k == 1))
            gate = wkd.tile([P, 256], DT_BF, tag="gate")
            nc.scalar.activation(gate[:], t2[:, 0:256], AF.Tanh,
                                 bias=_bcol(bp, "hbg2"), scale=0.5)
            prop = wkd.tile([P, 256], DT_F32, tag="prop")
            nc.vector.tensor_scalar_add(prop[:], t2[:, 256:512],
                                        _bcol(bp, "bp2"))
            rp2 = wkd.tile([P, 256], DT_BF, tag="rp2")
            nc.vector.tensor_scalar(rp2[:], t2[:, 256:512], _bcol(bp, "bp2"),
                                    0.0, op0=ALU.add, op1=ALU.max)
            t3 = pdb.tile([P, 512], DT_F32, tag="med")
            nc.tensor.matmul(t3[:, 0:256], wtb("tmu"), zp, start=True, stop=True)
            nc.tensor.matmul(t3[:, 256:512], wtb("tlv"), rp2[:],
                             start=True, stop=True)
            mub = wkd.tile([P, 256], DT_F32, tag="mub")
            nc.vector.tensor_scalar_add(mub[:], t3[:, 0:256], _bcol(bp, "btmu"))
            prlv = wkd.tile([P, 256], DT_F32, tag="prlv")
            nc.vector.tensor_scalar_add(prlv[:], t3[:, 256:512],
                                        _bcol(bp, "btlv"))
            # pr_mu = mub - 0.5*(dd + gate*dd),  dd = mub - prop
            dd = wkd.tile([P, 256], DT_F32, tag="dd")
            nc.gpsimd.tensor_tensor(dd[:], mub[:], prop[:], op=ALU.subtract)
            td = wkd.tile([P, 256], DT_F32, tag="td")
            nc.vector.tensor_tensor(td[:], gate[:], dd[:], op=ALU.mult)
            dts = wkd.tile([P, 256], DT_F32, tag="dts")
            nc.gpsimd.tensor_tensor(dts[:], dd[:], td[:], op=ALU.add)
            prmu = wkd.tile([P, 256], DT_F32, tag="prmu")
            nc.vector.scalar_tensor_tensor(prmu[:], dts[:], -0.5, mub[:],
                                           op0=ALU.mult, op1=ALU.add)
            # KL pieces: kl_elt = exp(lv-prlv) + (mu-prmu)^2*exp(-prlv) - (lv-prlv)
            a_ = wkd.tile([P, 256], DT_F32, tag="a")
            nc.gpsimd.tensor_tensor(a_[:], lv_t, prlv[:], op=ALU.subtract)
            e1 = wkd.tile([P, 256], DT_F32, tag="e1")
            nc.scalar.activation(e1[:], a_[:], AF.Exp)
            en = wkd.tile([P, 256], DT_F32, tag="en")
            nc.scalar.activation(en[:], prlv[:], AF.Exp, scale=-1.0)
            c_ = wkd.tile([P, 256], DT_F32, tag="c")
            nc.gpsimd.tensor_tensor(c_[:], mu_t, prmu[:], op=ALU.subtract)
            c2 = wkd.tile([P, 256], DT_F32, tag="c2")
            nc.gpsimd.tensor_tensor(c2[:], c_[:], c_[:], op=ALU.mult)
            t4 = wkd.tile([P, 256], DT_F32, tag="t4")
            nc.vector.tensor_tensor(t4[:], c2[:], en[:], op=ALU.mult)
            e1t = wkd.tile([P, 256], DT_F32, tag="e1t")
            nc.gpsimd.tensor_tensor(e1t[:], e1[:], t4[:], op=ALU.add)
            kscr = wkd.tile([P, 256], DT_F32, tag="kscr")
            nc.vector.scalar_tensor_tensor(kscr[:], e1t[:], 0.0, a_[:],
                                           op0=ALU.add, op1=ALU.subtract,
                                           accum_out=klS[:, n:n + 1])
            # ---- emitter + rec ----
            e12 = pda.tile([P, 1024], DT_F32, tag="big")
            for m in range(2):
                nc.tensor.matmul(e12[:, m * 256:(m + 1) * 256], wtb("eh1", m),
                                 zc, start=True, stop=True)
            re1 = wkd.tile([P, 512], DT_BF, tag="re1")
            nc.scalar.activation(re1[:, 0:256], e12[:, 0:256], AF.Relu,
                                 bias=_bcol(bp, "beh1", 0))
            nc.scalar.activation(re1[:, 256:512], e12[:, 256:512], AF.Relu,
                                 bias=_bcol(bp, "beh1", 1))
            for m in range(2):
                for k in range(2):
                    nc.tensor.matmul(e12[:, 512 + m * 256:512 + (m + 1) * 256],
                                     wtb("eh2", k * 2 + m),
                                     re1[:, k * 256:(k + 1) * 256],
                                     start=(k == 0), stop=(k == 1))
            re2 = wkd.tile([P, 512], DT_BF, tag="re2")
            nc.scalar.activation(re2[:, 0:256], e12[:, 512:768], AF.Relu,
                                 bias=_bcol(bp, "beh2", 0))
            nc.vector.tensor_scalar(re2[:, 256:512], e12[:, 768:1024],
                                    _bcol(bp, "beh2", 1), 0.0,
                                    op0=ALU.add, op1=ALU.max)
            e3 = pdb.tile([P, 512], DT_F32, tag="med")
            for k in range(2):
                nc.tensor.matmul(e3[:, 0:256], wtb("emu", k),
                                 re2[:, k * 256:(k + 1) * 256],
                                 start=(k == 0), stop=(k == 1))
            xmu = wkd.tile([P, 256], DT_BF, tag="xmu")
            nc.scalar.activation(xmu[:], e3[:, 0:256], AF.Identity,
                                 bias=_bcol(bp, "bemu"))
            esx = wkd.tile([P, 256], DT_BF, tag="esx")
            nc.vector.tensor_scalar(esx[:], ee_t[:], _bcol(bp, "escale"), None,
                                    op0=ALU.mult)
            d1 = wkd.tile([P, 256], DT_BF, tag="d1")
            nc.vector.tensor_tensor(d1[:], xmu[:], esx[:], op=ALU.add)
            d2 = wkd.tile([P, 256], DT_BF, tag="d2")
            nc.vector.tensor_tensor(d2[:], d1[:], y_t[:], op=ALU.subtract)
            rscr = wkd.tile([P, 256], DT_F32, tag="rscr")
            nc.scalar.activation(rscr[:], d2[:], AF.Square,
                                 accum_out=reS[:, n:n + 1])
            if dbg and n == 0:
                nc.sync.dma_start(dbg_aps["ytd"], y_t[:])
                nc.sync.dma_start(dbg_aps["d1"], d1[:])
                nc.sync.dma_start(dbg_aps["esx"], esx[:])
                nc.sync.dma_start(dbg_aps["d2"], d2[:])
                nc.sync.dma_start(dbg_aps["xmu"], xmu[:])
                nc.sync.dma_start(dbg_aps["re2"], re2[:])

        def phase_a(g):
            slot = g % 2
            xc = xs.tile([P, 512], DT_BF, tag="xc")
            nc.sync.dma_start(xc[:], xT[:, g * 512:(g + 1) * 512])
            for m in range(4):
                pa = pdb.tile([P, 512], DT_F32, tag="med")
                nc.tensor.matmul(pa[:], wtb("wih", m), xc[:],
                                 start=True, stop=True)
                dst = xwr[:, slot * 2048 + m * 512: slot * 2048 + (m + 1) * 512]
                if m % 2 == 0:
                    nc.scalar.activation(dst, pa[:], AF.Identity,
                                         bias=_bcol(bp, "brnn", m))
                else:
                    nc.vector.tensor_scalar_add(dst, pa[:],
                                                _bcol(bp, "brnn", m))

        def b_mm(t, pb, ms):
            hp = (t - 1) % 8
            for m in ms:
                for k in range(4):
                    if t == 0:
                        rhs = h0[:, k * 32:(k + 1) * 32]
                    else:
                        rhs = hr[:, hp * 128 + k * 32: hp * 128 + k * 32 + 32]
                    nc.tensor.matmul(pb[:, m * 32:(m + 1) * 32],
                                     wtb("whh", k * 4 + m), rhs,
                                     start=(k == 0), stop=(k == 3))

        def b_ew(t, pb):
            slot = (t // 16) % 2
            s = t % 16
            hs_ = t % 8
            tmpb = wk.tile([P, 128], DT_F32, tag="tmpb")
            nc.vector.tensor_tensor(
                tmpb[:].rearrange("p (m b) -> p m b", m=4),
                pb[:].rearrange("p (m b) -> p m b", m=4),
                xw3[:, slot, :, s, :], op=ALU.add)
            nc.scalar.activation(hr[:, hs_ * 128:(hs_ + 1) * 128],
                                 tmpb[:], AF.Relu)

        for _rep in range(reps):
          phase_a(0)
          pb = psb.tile([P, 128], DT_F32, tag="b")
          b_mm(0, pb, (0, 1, 2, 3))
          b_ew(0, pb)
          for g in range(G):
            slot = g % 2
            if g + 1 < G:
                phase_a(g + 1)
            ec = esp.tile([P, 512], DT_BF, tag="ec")
            nc.sync.dma_start(ec[:], epsC[:, g * 512:(g + 1) * 512])
            for s in range(16):
                t = 16 * g + s
                hs_ = t % 8
                has_next = t + 1 < T
                # C(t) combiner matmuls from z_{t-1}
                ph = psh.tile([P, 128], DT_F32, tag="hc")
                zprev = zal[:, t * 32:(t + 1) * 32]
                for m in range(4):
                    nc.tensor.matmul(ph[:, m * 32:(m + 1) * 32],
                                     wtb("cbh", m), zprev,
                                     start=True, stop=True)
                # B(t+1) first half fills the tanh window
                if has_next:
                    pb = psb.tile([P, 128], DT_F32, tag="b")
                    b_mm(t + 1, pb, (0, 1))
                # tanh path
                tmp2 = wk.tile([P, 128], DT_BF, tag="tmp2")
                if skip_cbh_bias:
                    # cb_h_b == 0: tanh straight from PSUM, no bias add
                    nc.scalar.activation(tmp2[:], ph[:], AF.Tanh)
                else:
                    tmp1 = wk.tile([P, 128], DT_F32, tag="tmp1")
                    nc.vector.tensor_tensor(tmp1[:], ph[:], bfl[:], op=ALU.add)
                    nc.scalar.activation(tmp2[:], tmp1[:], AF.Tanh)
                hc2 = wk.tile([P, 128], DT_BF, tag="hc2")
                nc.vector.tensor_tensor(hc2[:], tmp2[:],
                                        hr[:, hs_ * 128:(hs_ + 1) * 128],
                                        op=ALU.add)
                if has_next:
                    b_mm(t + 1, pb, (2, 3))
                # C(t) mu/lv matmuls (lv first so exp can start sooner)
                pm = psm.tile([P, 64], DT_F32, tag="ml")
                mlgroups = [(32, "cblv"), (0, "cbmu")] if lv_first else \
                           [(0, "cbmu"), (32, "cblv")]
                for off, wname in mlgroups:
                    for k in range(4):
                        nc.tensor.matmul(pm[:, off:off + 32], wtb(wname, k),
                                         hc2[:, k * 32:(k + 1) * 32],
                                         start=(k == 0), stop=(k == 3))
                if has_next:
                    b_ew(t + 1, pb)
                # z tail: z = (exp(lv/2)*eps + b_mu) + psum_mu
                ex = wk.tile([P, 32], DT_F32, tag="ex")
                nc.scalar.activation(ex[:], pm[:, 32:64], AF.Exp,
                                     bias=_bcol(bp, "hbcblv"), scale=0.5)
                t2t = wk.tile([P, 32], DT_F32, tag="t2t")
                nc.vector.tensor_tensor(t2t[:], ex[:],
                                        ec[:, s * 32:(s + 1) * 32],
                                        op=ALU.mult)
                nc.vector.scalar_tensor_tensor(
                    zal[:, (t + 1) * 32:(t + 2) * 32], t2t[:],
                    _bcol(bp, "bcbmu"), pm[:, 0:32],
                    op0=ALU.add, op1=ALU.add)
                # mu/lv staging for the KL pass (off the critical path)
                mu_sl = stg[:, slot * 1024 + s * 32: slot * 1024 + s * 32 + 32]
                lv_sl = stg[:, slot * 1024 + 512 + s * 32:
                            slot * 1024 + 512 + s * 32 + 32]
                if stage_dve:
                    nc.vector.tensor_scalar_add(mu_sl, pm[:, 0:32],
                                                _bcol(bp, "bcbmu"))
                    nc.vector.tensor_scalar_add(lv_sl, pm[:, 32:64],
                                                _bcol(bp, "bcblv"))
                else:
                    nc.scalar.activation(mu_sl, pm[:, 0:32], AF.Identity,
                                         bias=_bcol(bp, "bcbmu"))
                    nc.scalar.activation(lv_sl, pm[:, 32:64], AF.Identity,
                                         bias=_bcol(bp, "bcblv"))
                # first half-group's D chunk interleaves mid-group
                if s == d_split:
                    d_chunk(2 * g, slot, 0)
            # ---- second phase D chunk for this group ----
            if d_split >= 16:
                d_chunk(2 * g, slot, 0)
            d_chunk(2 * g + 1, slot, 1)

        # ---- final reduction ----
        sums2 = const.tile([P, 2], DT_F32)
        nc.vector.tensor_reduce(sums2[:, 0:1], reS[:], axis=AX.X, op=ALU.add)
        nc.vector.tensor_reduce(sums2[:, 1:2], klS[:], axis=AX.X, op=ALU.add)
        ones = const.tile([P, 1], DT_F32)
        nc.vector.memset(ones[:], 1.0)
        psf = psm.tile([1, 2], DT_F32, tag="ml")
        nc.tensor.matmul(psf[:], ones[:], sums2[:], start=True, stop=True)
        nc.vector.tensor_copy(fin[:], psf[:])
        nc.sync.dma_start(out_d, fin[:])
        if dbg:
            nc.sync.dma_start(dbg_aps["zal"], zal[:])
            nc.sync.dma_start(dbg_aps["stg"], stg[:])
            nc.sync.dma_start(dbg_aps["reS"], reS[:])
            nc.sync.dma_start(dbg_aps["klS"], klS[:])

    nc.compile()
    return nc


# ------------------------------------------------------------------
# host wrapper
# ------------------------------------------------------------------

def make_in_maps(inputs, T=T_FULL, n_cores=NCORES):
    wpk = _pack_weights(inputs)
    bpk = _pack_biases(inputs)
    bfu = _bfull(inputs)
    h0t = _h0t(inputs)
    z0t = np.broadcast_to(
        np.asarray(inputs["zq0"], np.float32)[:, None], (P, 32)
    ).astype(BF).copy()

    x = np.asarray(inputs["x"])[:, :T]
    y = np.asarray(inputs["y"])[:, :T]
    eC = np.asarray(inputs["eps_comb"])[:T]
    eE = np.asarray(inputs["eps_emit"])[:T]

    in_maps = []
    for c in range(n_cores):
        bs = slice(c * BC, (c + 1) * BC)
        xTc = np.ascontiguousarray(
            x[bs].transpose(2, 1, 0).reshape(P, T * BC)).astype(BF)
        yTc = np.ascontiguousarray(
            y[bs].transpose(2, 1, 0).reshape(P, T * BC)).astype(BF)
        eCc = np.ascontiguousarray(
            eC[:, bs, :].transpose(2, 0, 1).reshape(P, T * BC)).astype(BF)
        eEc = np.ascontiguousarray(
            eE[:, bs, :].transpose(2, 0, 1).reshape(P, T * BC)).astype(BF)
        in_maps.append(dict(xT=xTc, yT=yTc, epsC=eCc, epsE=eEc,
                            wpk=wpk, bpk=bpk, bfu=bfu, h0t=h0t, z0t=z0t))
    return in_maps


def combine_outputs(results, T=T_FULL, n_cores=None):
    if n_cores is None:
        n_cores = len(results)
    rec_sum = float(sum(r["res"][0, 0] for r in results))
    kl_sum = float(sum(r["res"][0, 1] for r in results))
    n_tb = T * BC * len(results)
    rec_loss = rec_sum / (n_tb * D)
    kl_loss = 0.5 * (kl_sum - n_tb * Z) / n_tb
    return np.array([rec_loss, kl_loss], np.float32)


_CACHE = {}


def kernel(**inputs):
    from concourse.bass_utils import run_bass_kernel_spmd
    T = T_FULL
    skip = not np.any(np.asarray(inputs["cb_h_b"]))
    key = (T, skip)
    if key not in _CACHE:
        _CACHE[key] = build_program(T, skip_cbh_bias=skip)
    nc = _CACHE[key]
    in_maps = make_in_maps(inputs, T)
    res = run_bass_kernel_spmd(nc, in_maps, core_ids=list(range(NCORES)))
    return combine_outputs(res.results, T)


if __name__ == "__main__":
    import jax
    inputs = None
    sys.path.insert(0, "/root/problem")
    import reference
    inputs = {k: np.asarray(v) for k, v in reference.setup_inputs().items()}
    out = kernel(**inputs)
    print("kernel out:", out)
